# revision 1
# baseline (speedup 1.0000x reference)
"""Trainium2 Bass kernel for the ChessTransformer problem.

Strategy: pure data-parallel over batch (B=2048 -> 256 samples/core on 8
NeuronCores). Activations are kept feature-major ([D on partitions, tokens on
free dim]) so every D-contraction feeds the PE directly. Attention is done
per-sample with a partition-axis softmax (no max subtraction needed; logits
are tiny by construction) using a ones-matmul to broadcast column sums.

The embedding gather is reformulated as a matmul: host builds a sparse
"count" matrix (one/two-hot over an 81-row table = 17 fen embeddings + 64
scaled position embeddings) and the device multiplies table^T @ counts.
"""

import sys

sys.path.insert(0, "/opt/trn_rl_repo")

import numpy as np
import ml_dtypes

import concourse.bacc as bacc
import concourse.bass as bass
import concourse.mybir as mybir
from concourse import tile
from concourse.bass_utils import run_bass_kernel_spmd

F32 = mybir.dt.float32
BF16 = mybir.dt.bfloat16
AF = mybir.ActivationFunctionType
ALU = mybir.AluOpType

D = 1024
H = 8
DH = 128
T = 71
KV = 81  # 17 fen rows + 64 pos rows
G = 7  # samples per compute tile (G*T = 497 <= 512 PSUM cols)
N_CORES = 8
EPS = 1e-5


def _bf(a):
    return np.ascontiguousarray(a.astype(ml_dtypes.bfloat16))


def _f32(a):
    return np.ascontiguousarray(a.astype(np.float32))


def host_prep(inputs, n_cores=N_CORES):
    """Build per-core input maps + flags from full-size inputs."""
    fen = np.asarray(inputs["fen"]).astype(np.int64)
    move = np.asarray(inputs["move"]).astype(np.int64)
    B = fen.shape[0]
    Bc = B // n_cores
    L = np.asarray(inputs["qkv"]).shape[0]

    rank_emb = np.asarray(inputs["rank_emb"], np.float32)
    file_emb = np.asarray(inputs["file_emb"], np.float32)
    fen_emb = np.asarray(inputs["fen_emb"], np.float32)
    move_emb = np.asarray(inputs["move_emb"], np.float32)
    abs_emb = np.asarray(inputs["abs_emb"], np.float32)
    qkv = np.asarray(inputs["qkv"], np.float32)
    ff1 = np.asarray(inputs["ff1"], np.float32)
    ff2 = np.asarray(inputs["ff2"], np.float32)
    W1 = np.asarray(inputs["W1"], np.float32)
    b1 = np.asarray(inputs["b1"], np.float32)
    W2 = np.asarray(inputs["W2"], np.float32)
    b2 = np.asarray(inputs["b2"], np.float32)
    lng = np.asarray(inputs["ln_emb_g"], np.float32)
    lnb = np.asarray(inputs["ln_emb_b"], np.float32)
    log = np.asarray(inputs["ln_out_g"], np.float32)
    lob = np.asarray(inputs["ln_out_b"], np.float32)

    pos = (rank_emb + file_emb).reshape(64, D)

    # table + per-token-position constants
    vtab = np.concatenate([fen_emb, 0.58 * pos], axis=0)  # (81, D)
    C = np.empty((T, D), np.float32)
    C[:64] = 0.5 * pos + abs_emb[:64]
    C[64:69] = abs_emb[64:69]
    C[69:71] = 0.58 * move_emb + abs_emb[69:71]

    # count matrix (two-hot embedding weights), cols = b*71 + t
    cnt = np.zeros((KV, B, T), np.float32)
    bidx = np.arange(B)[:, None]
    np.add.at(cnt, (fen[:, :64], bidx, np.arange(64)[None, :]), 0.5)
    np.add.at(cnt, (fen[:, 64:128], bidx, np.arange(64)[None, :]), 0.5)
    np.add.at(cnt, (fen[:, 128:133], bidx, np.arange(64, 69)[None, :]), 1.0)
    np.add.at(cnt, (17 + move, bidx, np.arange(69, 71)[None, :]), 1.0)
    cnt = cnt.reshape(KV, B * T)

    # const replicated G times: [8, 128, G*71]
    Cfm = C.T.reshape(8, 128, T)  # feature-major d-tiles
    cstr = np.tile(Cfm, (1, 1, G))

    scale = np.sqrt(np.float32(DH))
    wq = (qkv[:, 0] / scale).transpose(0, 2, 1, 3).reshape(L, 128, H * 128)
    wk = qkv[:, 1].transpose(0, 2, 1, 3).reshape(L, 128, H * 128)
    wv = qkv[:, 2].transpose(0, 2, 1, 3).reshape(L, 128, H * 128)
    wf1 = (
        ff1.reshape(L, H, 8, 128, DH).transpose(0, 3, 1, 2, 4).reshape(L, 128, H * 8 * 128)
    )
    wf2 = ff2.transpose(0, 2, 1, 3).reshape(L, 128, H * 128)

    w1t = W1.T.reshape(16, 128, 2 * D)  # [k, p, out]
    w2s = W2.reshape(16, 128).T  # [128, 16]
    hb1 = b1.reshape(16, 128).T  # [128, 16]
    hg = log.reshape(16, 128).T
    hbt = lob.reshape(16, 128).T
    gemb = lng.reshape(8, 128).T  # [128, 8]
    bemb = lnb.reshape(8, 128).T

    flags = dict(
        apply_gemb=not (np.all(lng == 1.0) and np.all(lnb == 0.0)),
        apply_ghead=not (np.all(log == 1.0) and np.all(lob == 0.0)),
        use_b1=bool(np.any(b1 != 0.0)),
        use_b2=bool(np.any(b2 != 0.0)),
        use_prelu=True,
        Bc=Bc,
        L=L,
    )

    shared = {
        "vtab": _bf(vtab),
        "cstr": _f32(cstr),
        "wq": _bf(wq),
        "wk": _bf(wk),
        "wv": _bf(wv),
        "wf1": _bf(wf1),
        "wf2": _bf(wf2),
        "w1t": _bf(w1t),
        "w2s": _bf(w2s),
        "hb1": _f32(hb1),
        "hb1s": _f32(0.2 * hb1),
        "hg": _f32(hg),
        "hbt": _f32(hbt),
        "gemb": _f32(gemb),
        "bemb": _f32(bemb),
        "b2": _f32(b2.reshape(1, 1)),
    }
    cnt_bf = _bf(cnt)
    in_maps = []
    for c in range(n_cores):
        m = dict(shared)
        m["cnt"] = np.ascontiguousarray(cnt_bf[:, c * Bc * T : (c + 1) * Bc * T])
        in_maps.append(m)
    return in_maps, flags


def build_program(flags):
    """Emit the full per-core program. Returns compiled-ready nc."""
    Bc = flags["Bc"]
    L = flags["L"]
    TOK = Bc * T
    NT = Bc // G  # full tiles
    REM = Bc - NT * G  # remainder samples
    NF = G * T  # 497
    NR = REM * T

    nc = bacc.Bacc("TRN2", target_bir_lowering=False, debug=False)

    cnt_d = nc.dram_tensor("cnt", [KV, TOK], BF16, kind="ExternalInput")
    vtab_d = nc.dram_tensor("vtab", [KV, D], BF16, kind="ExternalInput")
    cstr_d = nc.dram_tensor("cstr", [8, 128, NF], F32, kind="ExternalInput")
    wq_d = nc.dram_tensor("wq", [L, 128, H * 128], BF16, kind="ExternalInput")
    wk_d = nc.dram_tensor("wk", [L, 128, H * 128], BF16, kind="ExternalInput")
    wv_d = nc.dram_tensor("wv", [L, 128, H * 128], BF16, kind="ExternalInput")
    wf1_d = nc.dram_tensor("wf1", [L, 128, H * 8 * 128], BF16, kind="ExternalInput")
    wf2_d = nc.dram_tensor("wf2", [L, 128, H * 128], BF16, kind="ExternalInput")
    w1t_d = nc.dram_tensor("w1t", [16, 128, 2 * D], BF16, kind="ExternalInput")
    w2s_d = nc.dram_tensor("w2s", [128, 16], BF16, kind="ExternalInput")
    hb1_d = nc.dram_tensor("hb1", [128, 16], F32, kind="ExternalInput")
    hb1s_d = nc.dram_tensor("hb1s", [128, 16], F32, kind="ExternalInput")
    hg_d = nc.dram_tensor("hg", [128, 16], F32, kind="ExternalInput")
    hbt_d = nc.dram_tensor("hbt", [128, 16], F32, kind="ExternalInput")
    gemb_d = nc.dram_tensor("gemb", [128, 8], F32, kind="ExternalInput")
    bemb_d = nc.dram_tensor("bemb", [128, 8], F32, kind="ExternalInput")
    b2_d = nc.dram_tensor("b2", [1, 1], F32, kind="ExternalInput")
    out_d = nc.dram_tensor("out", [1, Bc], F32, kind="ExternalOutput")

    xa_d = nc.dram_tensor("xa", [8, 128, TOK], F32, kind="Internal")
    xb_d = nc.dram_tensor("xb", [8, 128, TOK], F32, kind="Internal")

    with tile.TileContext(nc) as tc:
        with tc.tile_pool(name="const", bufs=1) as cpool:
            ones71 = cpool.tile([71, 128], BF16)
            nc.vector.memset(ones71[:], 1.0)
            ones128 = cpool.tile([128, 128], BF16)
            nc.vector.memset(ones128[:], 1.0)
            epsT = cpool.tile([128, 1], F32)
            nc.vector.memset(epsT[:], EPS)
            al02 = cpool.tile([128, 1], F32)
            nc.vector.memset(al02[:], 0.2)

            def leaky(out_ap, in_ap, bias=0.0):
                # leaky_relu(in + bias) in one ACT op via Prelu (alpha AP);
                # falls back to scale-copy + max for CoreSim validation.
                if flags.get("use_prelu", True):
                    nc.scalar.activation(
                        out_ap, in_ap, AF.Prelu, bias=bias, alpha=al02[: in_ap.shape[0]]
                    )
                else:
                    t_ = cpool.tile([128, out_ap.shape[1]], F32, tag="lk")
                    p_ = t_[: in_ap.shape[0], :]
                    nc.scalar.activation(p_, in_ap, AF.Copy, scale=0.2)
                    if isinstance(bias, float):
                        nc.any.tensor_tensor(out_ap, p_, in_ap, ALU.max)
                    else:
                        s_ = cpool.tile([128, out_ap.shape[1]], F32, tag="lk2")
                        s2 = s_[: in_ap.shape[0], :]
                        nc.vector.tensor_scalar_add(s2, in_ap, bias)
                        nc.scalar.activation(p_, s2, AF.Copy, scale=0.2)
                        nc.any.tensor_tensor(out_ap, p_, s2, ALU.max)

            # ---------------- embedding ----------------
            with (
                tc.tile_pool(name="emb_sb", bufs=2) as esb,
                tc.tile_pool(name="emb_res", bufs=1) as eres,
                tc.tile_pool(name="emb_ps", bufs=2, space="PSUM") as eps_pool,
            ):
                vtab_sb = eres.tile([KV, D], BF16)
                nc.sync.dma_start(vtab_sb[:], vtab_d[:])
                cstr_sb = eres.tile([128, 8 * NF], F32)
                for k in range(8):
                    nc.sync.dma_start(cstr_sb[:, k * NF : (k + 1) * NF], cstr_d[k])
                if flags["apply_gemb"]:
                    gemb_sb = eres.tile([128, 8], F32)
                    nc.sync.dma_start(gemb_sb[:], gemb_d[:])
                    bemb_sb = eres.tile([128, 8], F32)
                    nc.sync.dma_start(bemb_sb[:], bemb_d[:])

                def embed_tile(cols, N):
                    cnt_t = esb.tile([KV, N], BF16, tag="cnt")
                    nc.sync.dma_start(cnt_t[:], cnt_d[:, cols])
                    xp = esb.tile([128, 8 * N], F32, tag="xp")
                    xbt = esb.tile([128, 8 * N], BF16, tag="xbt")
                    for k in range(8):
                        e_ps = eps_pool.tile([128, N], F32, tag="e")
                        nc.tensor.matmul(
                            e_ps[:], vtab_sb[:, k * 128 : (k + 1) * 128], cnt_t[:],
                            start=True, stop=True,
                        )
                        nc.any.tensor_tensor(
                            xp[:, k * N : (k + 1) * N], e_ps[:],
                            cstr_sb[:, k * NF : k * NF + N], ALU.add,
                        )
                        nc.any.tensor_copy(
                            xbt[:, k * N : (k + 1) * N], xp[:, k * N : (k + 1) * N]
                        )
                    mean_ps = eps_pool.tile([128, N], F32, tag="ln")
                    for k in range(8):
                        nc.tensor.matmul(
                            mean_ps[:], ones128[:], xbt[:, k * N : (k + 1) * N],
                            start=(k == 0), stop=(k == 7),
                        )
                    sq_ps = eps_pool.tile([128, N], F32, tag="ln")
                    for k in range(8):
                        sqt = esb.tile([128, N], BF16, tag="sq")
                        nc.scalar.activation(
                            sqt[:], xbt[:, k * N : (k + 1) * N], AF.Square
                        )
                        nc.tensor.matmul(
                            sq_ps[:], ones128[:], sqt[:],
                            start=(k == 0), stop=(k == 7),
                        )
                    m1 = esb.tile([128, N], F32, tag="m1")
                    nc.vector.tensor_scalar_mul(m1[:], mean_ps[:], 1.0 / D)
                    msq = esb.tile([128, N], F32, tag="msq")
                    nc.any.tensor_tensor(msq[:], m1[:], m1[:], ALU.mult)
                    v = esb.tile([128, N], F32, tag="v")
                    nc.vector.scalar_tensor_tensor(
                        v[:], sq_ps[:], 1.0 / D, msq[:], ALU.mult, ALU.subtract
                    )
                    s = esb.tile([128, N], F32, tag="s")
                    nc.scalar.activation(s[:], v[:], AF.Sqrt, bias=epsT[:])
                    r = esb.tile([128, N], F32, tag="r")
                    nc.vector.reciprocal(r[:], s[:])
                    for k in range(8):
                        xs = esb.tile([128, N], F32, tag="xs")
                        nc.any.tensor_tensor(
                            xs[:], xp[:, k * N : (k + 1) * N], m1[:], ALU.subtract
                        )
                        xn = esb.tile([128, N], F32, tag="xn")
                        nc.any.tensor_tensor(xn[:], xs[:], r[:], ALU.mult)
                        if flags["apply_gemb"]:
                            nc.vector.tensor_scalar(
                                xn[:], xn[:], gemb_sb[:, k : k + 1],
                                bemb_sb[:, k : k + 1], ALU.mult, ALU.add,
                            )
                        nc.sync.dma_start(xa_d[k][:, cols], xn[:])

                if NT > 0:
                    with tc.For_i(0, NT) as it:
                        embed_tile(bass.ts(it, NF), NF)
                if REM > 0:
                    off = NT * NF
                    embed_tile(slice(off, off + NR), NR)

            # ---------------- transformer layers ----------------
            with (
                tc.tile_pool(name="lw", bufs=2) as lw,
                tc.tile_pool(name="lsb", bufs=2) as lsb,
                tc.tile_pool(name="lbig", bufs=1) as lbig,
                tc.tile_pool(name="lps", bufs=1, space="PSUM") as lps,
            ):
                for l in range(L):
                    src, dst = (xa_d, xb_d) if l % 2 == 0 else (xb_d, xa_d)
                    wq_sb = lw.tile([128, H * 128], BF16, tag="wq")
                    nc.sync.dma_start(wq_sb[:], wq_d[l])
                    wk_sb = lw.tile([128, H * 128], BF16, tag="wk")
                    nc.sync.dma_start(wk_sb[:], wk_d[l])
                    wv_sb = lw.tile([128, H * 128], BF16, tag="wv")
                    nc.sync.dma_start(wv_sb[:], wv_d[l])
                    wf1_sb = lw.tile([128, H * 8 * 128], BF16, tag="wf1")
                    nc.sync.dma_start(wf1_sb[:], wf1_d[l])
                    wf2_sb = lw.tile([128, H * 128], BF16, tag="wf2")
                    nc.sync.dma_start(wf2_sb[:], wf2_d[l])

                    def layer_tile(cols, N, Gn):
                        xf = lbig.tile([128, 8 * N], F32, tag="xf")
                        for k in range(8):
                            nc.sync.dma_start(
                                xf[:, k * N : (k + 1) * N], src[k][:, cols]
                            )
                        xbt = lbig.tile([128, 8 * N], BF16, tag="xbt")
                        for k in range(8):
                            nc.any.tensor_copy(
                                xbt[:, k * N : (k + 1) * N], xf[:, k * N : (k + 1) * N]
                            )
                        xasb = lbig.tile([128, 8 * N], F32, tag="xasb")
                        xab = lbig.tile([128, 8 * N], BF16, tag="xab")
                        for h in range(8):
                            hs = slice(h * 128, (h + 1) * 128)
                            xh = xbt[:, h * N : (h + 1) * N]
                            q_ps = lps.tile([128, N], F32, tag="pj")
                            nc.tensor.matmul(q_ps[:], wq_sb[:, hs], xh, start=True, stop=True)
                            k_ps = lps.tile([128, N], F32, tag="pj2")
                            nc.tensor.matmul(k_ps[:], wk_sb[:, hs], xh, start=True, stop=True)
                            q_sb = lsb.tile([128, N], BF16, tag="q")
                            nc.any.tensor_copy(q_sb[:], q_ps[:])
                            k_sb = lsb.tile([128, N], BF16, tag="k")
                            nc.any.tensor_copy(k_sb[:], k_ps[:])
                            vt_ps = lps.tile([71, Gn * 128], F32, tag="vt")
                            for g in range(Gn):
                                nc.tensor.matmul(
                                    vt_ps[:, g * 128 : (g + 1) * 128],
                                    xh[:, g * T : g * T + T],
                                    wv_sb[:, hs],
                                    start=True, stop=True,
                                )
                            vt_sb = lsb.tile([71, Gn * 128], BF16, tag="vt")
                            nc.any.tensor_copy(vt_sb[:], vt_ps[:])
                            l_ps = lps.tile([71, N], F32, tag="att")
                            for g in range(Gn):
                                gs = slice(g * T, (g + 1) * T)
                                nc.tensor.matmul(
                                    l_ps[:, gs], k_sb[:, gs], q_sb[:, gs],
                                    start=True, stop=True,
                                )
                            el = lsb.tile([71, N], BF16, tag="el")
                            nc.scalar.activation(el[:], l_ps[:], AF.Exp)
                            # colsum broadcast onto 71 partitions; normalize el
                            # (bf16) before the AV matmul so y_ps is final.
                            cs_ps = lps.tile([71, N], F32, tag="att2")
                            nc.tensor.matmul(
                                cs_ps[:], ones71[:, :71], el[:], start=True, stop=True
                            )
                            r_sb = lsb.tile([71, N], F32, tag="r")
                            nc.vector.reciprocal(r_sb[:], cs_ps[:])
                            eln = lsb.tile([71, N], BF16, tag="eln")
                            nc.vector.tensor_tensor(eln[:], el[:], r_sb[:], ALU.mult)
                            y_ps = lps.tile([128, N], F32, tag="att3")
                            for g in range(Gn):
                                gs = slice(g * T, (g + 1) * T)
                                nc.tensor.matmul(
                                    y_ps[:, gs], vt_sb[:, g * 128 : (g + 1) * 128],
                                    eln[:, gs], start=True, stop=True,
                                )
                            nc.any.tensor_tensor(
                                xasb[:, h * N : (h + 1) * N], y_ps[:],
                                xf[:, h * N : (h + 1) * N], ALU.add,
                            )
                            nc.any.tensor_copy(
                                xab[:, h * N : (h + 1) * N], xasb[:, h * N : (h + 1) * N]
                            )
                        y1 = lbig.tile([128, 8 * N], BF16, tag="y1")
                        for m in range(8):
                            f_ps = lps.tile([128, N], F32, tag="pj")
                            for k in range(8):
                                nc.tensor.matmul(
                                    f_ps[:],
                                    wf1_sb[:, (m * 8 + k) * 128 : (m * 8 + k + 1) * 128],
                                    xab[:, k * N : (k + 1) * N],
                                    start=(k == 0), stop=(k == 7),
                                )
                            leaky(y1[:, m * N : (m + 1) * N], f_ps[:])
                        for h in range(8):
                            g_ps = lps.tile([128, N], F32, tag="att2")
                            nc.tensor.matmul(
                                g_ps[:], wf2_sb[:, h * 128 : (h + 1) * 128],
                                y1[:, h * N : (h + 1) * N], start=True, stop=True,
                            )
                            y2 = lsb.tile([128, N], F32, tag="y2")
                            leaky(y2[:], g_ps[:])
                            xo = lsb.tile([128, N], F32, tag="xo")
                            nc.any.tensor_tensor(
                                xo[:], y2[:], xasb[:, h * N : (h + 1) * N], ALU.add
                            )
                            nc.sync.dma_start(dst[h][:, cols], xo[:])

                    if NT > 0:
                        with tc.For_i(0, NT) as it:
                            layer_tile(bass.ts(it, NF), NF, G)
                    if REM > 0:
                        off = NT * NF
                        layer_tile(slice(off, off + NR), NR, REM)

            # ---------------- head ----------------
            xfin = xa_d if L % 2 == 0 else xb_d
            with (
                tc.tile_pool(name="h_sb", bufs=2) as hsb,
                tc.tile_pool(name="h_res", bufs=1) as hres,
                tc.tile_pool(name="h_ps", bufs=2, space="PSUM") as hps,
            ):
                u = hres.tile([128, 16 * Bc], F32)
                for k in range(16):
                    tok = 69 if k < 8 else 70
                    srcap = xfin[k % 8].rearrange("p (b t) -> p b t", t=T)[:, :, tok]
                    nc.sync.dma_start(u[:, k * Bc : (k + 1) * Bc], srcap)
                ub = hres.tile([128, 16 * Bc], BF16)
                for k in range(16):
                    ks = slice(k * Bc, (k + 1) * Bc)
                    nc.any.tensor_copy(ub[:, ks], u[:, ks])
                mean_ps = hps.tile([128, Bc], F32, tag="ln")
                for k in range(16):
                    nc.tensor.matmul(
                        mean_ps[:], ones128[:], ub[:, k * Bc : (k + 1) * Bc],
                        start=(k == 0), stop=(k == 15),
                    )
                sq_ps = hps.tile([128, Bc], F32, tag="ln")
                for k in range(16):
                    sqt = hsb.tile([128, Bc], BF16, tag="sq")
                    nc.scalar.activation(sqt[:], ub[:, k * Bc : (k + 1) * Bc], AF.Square)
                    nc.tensor.matmul(
                        sq_ps[:], ones128[:], sqt[:], start=(k == 0), stop=(k == 15)
                    )
                m1 = hsb.tile([128, Bc], F32, tag="m1")
                nc.vector.tensor_scalar_mul(m1[:], mean_ps[:], 1.0 / (2 * D))
                msq = hsb.tile([128, Bc], F32, tag="msq")
                nc.any.tensor_tensor(msq[:], m1[:], m1[:], ALU.mult)
                v = hsb.tile([128, Bc], F32, tag="v")
                nc.vector.scalar_tensor_tensor(
                    v[:], sq_ps[:], 1.0 / (2 * D), msq[:], ALU.mult, ALU.subtract
                )
                s = hsb.tile([128, Bc], F32, tag="s")
                nc.scalar.activation(s[:], v[:], AF.Sqrt, bias=epsT[:])
                r = hsb.tile([128, Bc], F32, tag="r")
                nc.vector.reciprocal(r[:], s[:])
                if flags["apply_ghead"]:
                    hg_sb = hres.tile([128, 16], F32)
                    nc.sync.dma_start(hg_sb[:], hg_d[:])
                    hbt_sb = hres.tile([128, 16], F32)
                    nc.sync.dma_start(hbt_sb[:], hbt_d[:])
                unb = hres.tile([128, 16 * Bc], BF16)
                for k in range(16):
                    ks = slice(k * Bc, (k + 1) * Bc)
                    xs = hsb.tile([128, Bc], F32, tag="xs")
                    nc.any.tensor_tensor(xs[:], u[:, ks], m1[:], ALU.subtract)
                    xn = hsb.tile([128, Bc], F32, tag="xn")
                    nc.any.tensor_tensor(xn[:], xs[:], r[:], ALU.mult)
                    if flags["apply_ghead"]:
                        nc.vector.tensor_scalar(
                            xn[:], xn[:], hg_sb[:, k : k + 1], hbt_sb[:, k : k + 1],
                            ALU.mult, ALU.add,
                        )
                    nc.any.tensor_copy(unb[:, ks], xn[:])
                # W1 + leaky
                w1_tiles = []
                for k in range(16):
                    wt = hres.tile([128, 2 * D], BF16, tag=f"w1_{k}")
                    nc.sync.dma_start(wt[:], w1t_d[k])
                    w1_tiles.append(wt)
                if flags["use_b1"]:
                    hb1_sb = hres.tile([128, 16], F32)
                    nc.sync.dma_start(hb1_sb[:], hb1_d[:])
                    hb1s_sb = hres.tile([128, 16], F32)
                    nc.sync.dma_start(hb1s_sb[:], hb1s_d[:])
                h1 = hres.tile([128, 16 * Bc], BF16)
                for m in range(16):
                    f_ps = hps.tile([128, Bc], F32, tag="f")
                    for k in range(16):
                        nc.tensor.matmul(
                            f_ps[:], w1_tiles[k][:, m * 128 : (m + 1) * 128],
                            unb[:, k * Bc : (k + 1) * Bc],
                            start=(k == 0), stop=(k == 15),
                        )
                    t1 = hsb.tile([128, Bc], F32, tag="t1")
                    ms = slice(m * Bc, (m + 1) * Bc)
                    if flags["use_b1"]:
                        nc.scalar.activation(
                            t1[:], f_ps[:], AF.Copy, scale=0.2, bias=hb1s_sb[:, m : m + 1]
                        )
                        s1 = hsb.tile([128, Bc], F32, tag="s1")
                        nc.vector.tensor_scalar_add(s1[:], f_ps[:], hb1_sb[:, m : m + 1])
                        nc.any.tensor_tensor(h1[:, ms], t1[:], s1[:], ALU.max)
                    else:
                        nc.scalar.activation(t1[:], f_ps[:], AF.Copy, scale=0.2)
                        nc.any.tensor_tensor(h1[:, ms], t1[:], f_ps[:], ALU.max)
                w2_sb = hres.tile([128, 16], BF16)
                nc.sync.dma_start(w2_sb[:], w2s_d[:])
                o_ps = hps.tile([1, Bc], F32, tag="o")
                for k in range(16):
                    nc.tensor.matmul(
                        o_ps[:], w2_sb[:, k : k + 1], h1[:, k * Bc : (k + 1) * Bc],
                        start=(k == 0), stop=(k == 15),
                    )
                o_sb = hsb.tile([1, Bc], F32, tag="o")
                if flags["use_b2"]:
                    b2_sb = hres.tile([1, 1], F32)
                    nc.sync.dma_start(b2_sb[:], b2_d[:])
                    nc.scalar.activation(o_sb[:], o_ps[:], AF.Sigmoid, bias=b2_sb[:])
                else:
                    nc.scalar.activation(o_sb[:], o_ps[:], AF.Sigmoid)
                nc.sync.dma_start(out_d[:], o_sb[:])

    return nc


TRACE = False
LAST_RESULT = None


def kernel(**inputs):
    global LAST_RESULT
    in_maps, flags = host_prep(inputs, N_CORES)
    nc = build_program(flags)
    nc.compile()
    res = run_bass_kernel_spmd(
        nc, in_maps, core_ids=list(range(N_CORES)), trace=TRACE
    )
    LAST_RESULT = res
    Bc = flags["Bc"]
    out = np.concatenate([res.results[c]["out"].reshape(Bc, 1) for c in range(N_CORES)])
    return out.astype(np.float32)



# revision 3
# speedup vs baseline: 1.3132x; 1.3132x over previous
"""Trainium2 Bass kernel for the ChessTransformer problem.

Strategy: pure data-parallel over batch (B=2048 -> 256 samples/core on 8
NeuronCores). Activations are kept feature-major ([D on partitions, tokens on
free dim]) so every D-contraction feeds the PE directly. Attention is done
per-sample with a partition-axis softmax (no max subtraction needed; logits
are tiny by construction) using a ones-matmul to broadcast column sums.

The embedding gather is reformulated as a matmul: host builds a sparse
"count" matrix (one/two-hot over an 81-row table = 17 fen embeddings + 64
scaled position embeddings) and the device multiplies table^T @ counts.
"""

import sys

sys.path.insert(0, "/opt/trn_rl_repo")

import numpy as np
import ml_dtypes

import concourse.bacc as bacc
import concourse.bass as bass
import concourse.mybir as mybir
from concourse import tile
from concourse.bass_utils import run_bass_kernel_spmd

F32 = mybir.dt.float32
BF16 = mybir.dt.bfloat16
AF = mybir.ActivationFunctionType
ALU = mybir.AluOpType

D = 1024
H = 8
DH = 128
T = 71
KV = 81  # 17 fen rows + 64 pos rows
G = 7  # samples per compute tile (G*T = 497 <= 512 PSUM cols)
N_CORES = 8
EPS = 1e-5


def _bf(a):
    return np.ascontiguousarray(a.astype(ml_dtypes.bfloat16))


def _f32(a):
    return np.ascontiguousarray(a.astype(np.float32))


def host_prep(inputs, n_cores=N_CORES):
    """Build per-core input maps + flags from full-size inputs."""
    fen = np.asarray(inputs["fen"]).astype(np.int64)
    move = np.asarray(inputs["move"]).astype(np.int64)
    B = fen.shape[0]
    Bc = B // n_cores
    L = np.asarray(inputs["qkv"]).shape[0]

    rank_emb = np.asarray(inputs["rank_emb"], np.float32)
    file_emb = np.asarray(inputs["file_emb"], np.float32)
    fen_emb = np.asarray(inputs["fen_emb"], np.float32)
    move_emb = np.asarray(inputs["move_emb"], np.float32)
    abs_emb = np.asarray(inputs["abs_emb"], np.float32)
    qkv = np.asarray(inputs["qkv"], np.float32)
    ff1 = np.asarray(inputs["ff1"], np.float32)
    ff2 = np.asarray(inputs["ff2"], np.float32)
    W1 = np.asarray(inputs["W1"], np.float32)
    b1 = np.asarray(inputs["b1"], np.float32)
    W2 = np.asarray(inputs["W2"], np.float32)
    b2 = np.asarray(inputs["b2"], np.float32)
    lng = np.asarray(inputs["ln_emb_g"], np.float32)
    lnb = np.asarray(inputs["ln_emb_b"], np.float32)
    log = np.asarray(inputs["ln_out_g"], np.float32)
    lob = np.asarray(inputs["ln_out_b"], np.float32)

    pos = (rank_emb + file_emb).reshape(64, D)

    # table + per-token-position constants
    vtab = np.concatenate([fen_emb, 0.58 * pos], axis=0)  # (81, D)
    C = np.empty((T, D), np.float32)
    C[:64] = 0.5 * pos + abs_emb[:64]
    C[64:69] = abs_emb[64:69]
    C[69:71] = 0.58 * move_emb + abs_emb[69:71]

    # count matrix (two-hot embedding weights), cols = b*71 + t
    cnt = np.zeros((KV, B, T), np.float32)
    bidx = np.arange(B)[:, None]
    np.add.at(cnt, (fen[:, :64], bidx, np.arange(64)[None, :]), 0.5)
    np.add.at(cnt, (fen[:, 64:128], bidx, np.arange(64)[None, :]), 0.5)
    np.add.at(cnt, (fen[:, 128:133], bidx, np.arange(64, 69)[None, :]), 1.0)
    np.add.at(cnt, (17 + move, bidx, np.arange(69, 71)[None, :]), 1.0)
    cnt = cnt.reshape(KV, B * T)

    # const replicated G times: [8, 128, G*71]
    Cfm = C.T.reshape(8, 128, T)  # feature-major d-tiles
    cstr = np.tile(Cfm, (1, 1, G))

    scale = np.sqrt(np.float32(DH))
    wq = (qkv[:, 0] / scale).transpose(0, 2, 1, 3).reshape(L, 128, H * 128)
    wk = qkv[:, 1].transpose(0, 2, 1, 3).reshape(L, 128, H * 128)
    wv = qkv[:, 2].transpose(0, 2, 1, 3).reshape(L, 128, H * 128)
    wf1 = (
        ff1.reshape(L, H, 8, 128, DH).transpose(0, 3, 1, 2, 4).reshape(L, 128, H * 8 * 128)
    )
    wf2 = ff2.transpose(0, 2, 1, 3).reshape(L, 128, H * 128)

    w1t = W1.T.reshape(16, 128, 2 * D)  # [k, p, out]
    w2s = W2.reshape(16, 128).T  # [128, 16]
    hb1 = b1.reshape(16, 128).T  # [128, 16]
    hg = log.reshape(16, 128).T
    hbt = lob.reshape(16, 128).T
    gemb = lng.reshape(8, 128).T  # [128, 8]
    bemb = lnb.reshape(8, 128).T

    flags = dict(
        apply_gemb=not (np.all(lng == 1.0) and np.all(lnb == 0.0)),
        apply_ghead=not (np.all(log == 1.0) and np.all(lob == 0.0)),
        use_b1=bool(np.any(b1 != 0.0)),
        use_b2=bool(np.any(b2 != 0.0)),
        use_prelu=True,
        Bc=Bc,
        L=L,
    )

    shared = {
        "vtab": _bf(vtab),
        "cstr": _f32(cstr),
        "wq": _bf(wq),
        "wk": _bf(wk),
        "wv": _bf(wv),
        "wf1": _bf(wf1),
        "wf2": _bf(wf2),
        "w1t": _bf(w1t),
        "w2s": _bf(w2s),
        "hb1": _f32(hb1),
        "hb1s": _f32(0.2 * hb1),
        "hg": _f32(hg),
        "hbt": _f32(hbt),
        "gemb": _f32(gemb),
        "bemb": _f32(bemb),
        "b2": _f32(b2.reshape(1, 1)),
    }
    cnt_bf = _bf(cnt)
    in_maps = []
    for c in range(n_cores):
        m = dict(shared)
        m["cnt"] = np.ascontiguousarray(cnt_bf[:, c * Bc * T : (c + 1) * Bc * T])
        in_maps.append(m)
    return in_maps, flags


def build_program(flags):
    """Emit the full per-core program. Returns compiled-ready nc."""
    Bc = flags["Bc"]
    L = flags["L"]
    TOK = Bc * T
    NT = Bc // G  # full tiles
    REM = Bc - NT * G  # remainder samples
    NF = G * T  # 497
    NR = REM * T

    nc = bacc.Bacc("TRN2", target_bir_lowering=False, debug=False)

    cnt_d = nc.dram_tensor("cnt", [KV, TOK], BF16, kind="ExternalInput")
    vtab_d = nc.dram_tensor("vtab", [KV, D], BF16, kind="ExternalInput")
    cstr_d = nc.dram_tensor("cstr", [8, 128, NF], F32, kind="ExternalInput")
    wq_d = nc.dram_tensor("wq", [L, 128, H * 128], BF16, kind="ExternalInput")
    wk_d = nc.dram_tensor("wk", [L, 128, H * 128], BF16, kind="ExternalInput")
    wv_d = nc.dram_tensor("wv", [L, 128, H * 128], BF16, kind="ExternalInput")
    wf1_d = nc.dram_tensor("wf1", [L, 128, H * 8 * 128], BF16, kind="ExternalInput")
    wf2_d = nc.dram_tensor("wf2", [L, 128, H * 128], BF16, kind="ExternalInput")
    w1t_d = nc.dram_tensor("w1t", [16, 128, 2 * D], BF16, kind="ExternalInput")
    w2s_d = nc.dram_tensor("w2s", [128, 16], BF16, kind="ExternalInput")
    hb1_d = nc.dram_tensor("hb1", [128, 16], F32, kind="ExternalInput")
    hb1s_d = nc.dram_tensor("hb1s", [128, 16], F32, kind="ExternalInput")
    hg_d = nc.dram_tensor("hg", [128, 16], F32, kind="ExternalInput")
    hbt_d = nc.dram_tensor("hbt", [128, 16], F32, kind="ExternalInput")
    gemb_d = nc.dram_tensor("gemb", [128, 8], F32, kind="ExternalInput")
    bemb_d = nc.dram_tensor("bemb", [128, 8], F32, kind="ExternalInput")
    b2_d = nc.dram_tensor("b2", [1, 1], F32, kind="ExternalInput")
    out_d = nc.dram_tensor("out", [1, Bc], F32, kind="ExternalOutput")

    xa_d = nc.dram_tensor("xa", [8, 128, TOK], F32, kind="Internal")
    xb_d = nc.dram_tensor("xb", [8, 128, TOK], F32, kind="Internal")

    with tile.TileContext(nc) as tc:
        with tc.tile_pool(name="const", bufs=1) as cpool:
            ones71 = cpool.tile([71, 128], BF16)
            nc.vector.memset(ones71[:], 1.0)
            ones128 = cpool.tile([128, 128], BF16)
            nc.vector.memset(ones128[:], 1.0)
            epsT = cpool.tile([128, 1], F32)
            nc.vector.memset(epsT[:], EPS)
            al02 = cpool.tile([128, 1], F32)
            nc.vector.memset(al02[:], 0.2)

            def leaky(out_ap, in_ap, bias=0.0):
                # leaky_relu(in + bias) in one ACT op via Prelu (alpha AP);
                # falls back to scale-copy + max for CoreSim validation.
                if flags.get("use_prelu", True):
                    nc.scalar.activation(
                        out_ap, in_ap, AF.Prelu, bias=bias, alpha=al02[: in_ap.shape[0]]
                    )
                else:
                    t_ = cpool.tile([128, out_ap.shape[1]], F32, tag="lk")
                    p_ = t_[: in_ap.shape[0], :]
                    nc.scalar.activation(p_, in_ap, AF.Copy, scale=0.2)
                    if isinstance(bias, float):
                        nc.any.tensor_tensor(out_ap, p_, in_ap, ALU.max)
                    else:
                        s_ = cpool.tile([128, out_ap.shape[1]], F32, tag="lk2")
                        s2 = s_[: in_ap.shape[0], :]
                        nc.vector.tensor_scalar_add(s2, in_ap, bias)
                        nc.scalar.activation(p_, s2, AF.Copy, scale=0.2)
                        nc.any.tensor_tensor(out_ap, p_, s2, ALU.max)

            # ---------------- embedding ----------------
            with (
                tc.tile_pool(name="emb_sb", bufs=2) as esb,
                tc.tile_pool(name="emb_res", bufs=1) as eres,
                tc.tile_pool(name="emb_ps", bufs=2, space="PSUM") as eps_pool,
            ):
                vtab_sb = eres.tile([KV, D], BF16)
                nc.sync.dma_start(vtab_sb[:], vtab_d[:])
                cstr_sb = eres.tile([128, 8 * NF], F32)
                for k in range(8):
                    nc.sync.dma_start(cstr_sb[:, k * NF : (k + 1) * NF], cstr_d[k])
                if flags["apply_gemb"]:
                    gemb_sb = eres.tile([128, 8], F32)
                    nc.sync.dma_start(gemb_sb[:], gemb_d[:])
                    bemb_sb = eres.tile([128, 8], F32)
                    nc.sync.dma_start(bemb_sb[:], bemb_d[:])

                def embed_tile(cols, N):
                    cnt_t = esb.tile([KV, N], BF16, tag="cnt")
                    nc.sync.dma_start(cnt_t[:], cnt_d[:, cols])
                    xp = esb.tile([128, 8 * N], F32, tag="xp")
                    xbt = esb.tile([128, 8 * N], BF16, tag="xbt")
                    for k in range(8):
                        e_ps = eps_pool.tile([128, N], F32, tag="e")
                        nc.tensor.matmul(
                            e_ps[:], vtab_sb[:, k * 128 : (k + 1) * 128], cnt_t[:],
                            start=True, stop=True,
                        )
                        nc.any.tensor_tensor(
                            xp[:, k * N : (k + 1) * N], e_ps[:],
                            cstr_sb[:, k * NF : k * NF + N], ALU.add,
                        )
                        nc.any.tensor_copy(
                            xbt[:, k * N : (k + 1) * N], xp[:, k * N : (k + 1) * N]
                        )
                    mean_ps = eps_pool.tile([128, N], F32, tag="ln")
                    for k in range(8):
                        nc.tensor.matmul(
                            mean_ps[:], ones128[:], xbt[:, k * N : (k + 1) * N],
                            start=(k == 0), stop=(k == 7),
                        )
                    sq_ps = eps_pool.tile([128, N], F32, tag="ln")
                    for k in range(8):
                        sqt = esb.tile([128, N], BF16, tag="sq")
                        nc.scalar.activation(
                            sqt[:], xbt[:, k * N : (k + 1) * N], AF.Square
                        )
                        nc.tensor.matmul(
                            sq_ps[:], ones128[:], sqt[:],
                            start=(k == 0), stop=(k == 7),
                        )
                    m1 = esb.tile([128, N], F32, tag="m1")
                    nc.vector.tensor_scalar_mul(m1[:], mean_ps[:], 1.0 / D)
                    msq = esb.tile([128, N], F32, tag="msq")
                    nc.any.tensor_tensor(msq[:], m1[:], m1[:], ALU.mult)
                    v = esb.tile([128, N], F32, tag="v")
                    nc.vector.scalar_tensor_tensor(
                        v[:], sq_ps[:], 1.0 / D, msq[:], ALU.mult, ALU.subtract
                    )
                    s = esb.tile([128, N], F32, tag="s")
                    nc.scalar.activation(s[:], v[:], AF.Sqrt, bias=epsT[:])
                    r = esb.tile([128, N], F32, tag="r")
                    nc.vector.reciprocal(r[:], s[:])
                    for k in range(8):
                        xs = esb.tile([128, N], F32, tag="xs")
                        nc.any.tensor_tensor(
                            xs[:], xp[:, k * N : (k + 1) * N], m1[:], ALU.subtract
                        )
                        xn = esb.tile([128, N], F32, tag="xn")
                        nc.any.tensor_tensor(xn[:], xs[:], r[:], ALU.mult)
                        if flags["apply_gemb"]:
                            nc.vector.tensor_scalar(
                                xn[:], xn[:], gemb_sb[:, k : k + 1],
                                bemb_sb[:, k : k + 1], ALU.mult, ALU.add,
                            )
                        nc.sync.dma_start(xa_d[k][:, cols], xn[:])

                if NT > 0:
                    if flags.get("unroll"):
                        for it in range(NT):
                            embed_tile(slice(it * NF, (it + 1) * NF), NF)
                    else:
                        with tc.For_i(0, NT) as it:
                            embed_tile(bass.ts(it, NF), NF)
                if REM > 0:
                    off = NT * NF
                    embed_tile(slice(off, off + NR), NR)

            # ---------------- transformer layers ----------------
            with (
                tc.tile_pool(name="lw", bufs=2) as lw,
                tc.tile_pool(name="lsb", bufs=2) as lsb,
                tc.tile_pool(name="lbig", bufs=1) as lbig,
                tc.tile_pool(name="lps", bufs=1, space="PSUM") as lps,
            ):
                for l in range(L):
                    src, dst = (xa_d, xb_d) if l % 2 == 0 else (xb_d, xa_d)
                    wq_sb = lw.tile([128, H * 128], BF16, tag="wq")
                    nc.sync.dma_start(wq_sb[:], wq_d[l])
                    wk_sb = lw.tile([128, H * 128], BF16, tag="wk")
                    nc.sync.dma_start(wk_sb[:], wk_d[l])
                    wv_sb = lw.tile([128, H * 128], BF16, tag="wv")
                    nc.sync.dma_start(wv_sb[:], wv_d[l])
                    wf1_sb = lw.tile([128, H * 8 * 128], BF16, tag="wf1")
                    nc.sync.dma_start(wf1_sb[:], wf1_d[l])
                    wf2_sb = lw.tile([128, H * 128], BF16, tag="wf2")
                    nc.sync.dma_start(wf2_sb[:], wf2_d[l])

                    def layer_tile(cols, N, Gn):
                        xf = lbig.tile([128, 8 * N], F32, tag="xf")
                        for k in range(8):
                            nc.sync.dma_start(
                                xf[:, k * N : (k + 1) * N], src[k][:, cols]
                            )
                        xbt = lbig.tile([128, 8 * N], BF16, tag="xbt")
                        for k in range(8):
                            nc.any.tensor_copy(
                                xbt[:, k * N : (k + 1) * N], xf[:, k * N : (k + 1) * N]
                            )
                        xasb = lbig.tile([128, 8 * N], F32, tag="xasb")
                        xab = lbig.tile([128, 8 * N], BF16, tag="xab")
                        for h in range(8):
                            hs = slice(h * 128, (h + 1) * 128)
                            xh = xbt[:, h * N : (h + 1) * N]
                            q_ps = lps.tile([128, N], F32, tag="pj")
                            nc.tensor.matmul(q_ps[:], wq_sb[:, hs], xh, start=True, stop=True)
                            k_ps = lps.tile([128, N], F32, tag="pj2")
                            nc.tensor.matmul(k_ps[:], wk_sb[:, hs], xh, start=True, stop=True)
                            q_sb = lsb.tile([128, N], BF16, tag="q")
                            nc.any.tensor_copy(q_sb[:], q_ps[:])
                            k_sb = lsb.tile([128, N], BF16, tag="k")
                            nc.any.tensor_copy(k_sb[:], k_ps[:])
                            vt_ps = lps.tile([71, Gn * 128], F32, tag="vt")
                            for g in range(Gn):
                                nc.tensor.matmul(
                                    vt_ps[:, g * 128 : (g + 1) * 128],
                                    xh[:, g * T : g * T + T],
                                    wv_sb[:, hs],
                                    start=True, stop=True,
                                )
                            vt_sb = lsb.tile([71, Gn * 128], BF16, tag="vt")
                            nc.any.tensor_copy(vt_sb[:], vt_ps[:])
                            l_ps = lps.tile([71, N], F32, tag="att")
                            for g in range(Gn):
                                gs = slice(g * T, (g + 1) * T)
                                nc.tensor.matmul(
                                    l_ps[:, gs], k_sb[:, gs], q_sb[:, gs],
                                    start=True, stop=True,
                                )
                            el = lsb.tile([71, N], BF16, tag="el")
                            nc.scalar.activation(el[:], l_ps[:], AF.Exp)
                            # colsum broadcast onto 71 partitions; normalize el
                            # (bf16) before the AV matmul so y_ps is final.
                            cs_ps = lps.tile([71, N], F32, tag="att2")
                            nc.tensor.matmul(
                                cs_ps[:], ones71[:, :71], el[:], start=True, stop=True
                            )
                            r_sb = lsb.tile([71, N], F32, tag="r")
                            nc.vector.reciprocal(r_sb[:], cs_ps[:])
                            eln = lsb.tile([71, N], BF16, tag="eln")
                            nc.vector.tensor_tensor(eln[:], el[:], r_sb[:], ALU.mult)
                            y_ps = lps.tile([128, N], F32, tag="att3")
                            for g in range(Gn):
                                gs = slice(g * T, (g + 1) * T)
                                nc.tensor.matmul(
                                    y_ps[:, gs], vt_sb[:, g * 128 : (g + 1) * 128],
                                    eln[:, gs], start=True, stop=True,
                                )
                            nc.any.tensor_tensor(
                                xasb[:, h * N : (h + 1) * N], y_ps[:],
                                xf[:, h * N : (h + 1) * N], ALU.add,
                            )
                            nc.any.tensor_copy(
                                xab[:, h * N : (h + 1) * N], xasb[:, h * N : (h + 1) * N]
                            )
                        y1 = lbig.tile([128, 8 * N], BF16, tag="y1")
                        for m in range(8):
                            f_ps = lps.tile([128, N], F32, tag="pj")
                            for k in range(8):
                                nc.tensor.matmul(
                                    f_ps[:],
                                    wf1_sb[:, (m * 8 + k) * 128 : (m * 8 + k + 1) * 128],
                                    xab[:, k * N : (k + 1) * N],
                                    start=(k == 0), stop=(k == 7),
                                )
                            leaky(y1[:, m * N : (m + 1) * N], f_ps[:])
                        for h in range(8):
                            g_ps = lps.tile([128, N], F32, tag="att2")
                            nc.tensor.matmul(
                                g_ps[:], wf2_sb[:, h * 128 : (h + 1) * 128],
                                y1[:, h * N : (h + 1) * N], start=True, stop=True,
                            )
                            y2 = lsb.tile([128, N], F32, tag="y2")
                            leaky(y2[:], g_ps[:])
                            xo = lsb.tile([128, N], F32, tag="xo")
                            nc.any.tensor_tensor(
                                xo[:], y2[:], xasb[:, h * N : (h + 1) * N], ALU.add
                            )
                            nc.sync.dma_start(dst[h][:, cols], xo[:])

                    if NT > 0:
                        if flags.get("unroll"):
                            for it in range(NT):
                                layer_tile(slice(it * NF, (it + 1) * NF), NF, G)
                        else:
                            with tc.For_i(0, NT) as it:
                                layer_tile(bass.ts(it, NF), NF, G)
                    if REM > 0:
                        off = NT * NF
                        layer_tile(slice(off, off + NR), NR, REM)

            # ---------------- head ----------------
            xfin = xa_d if L % 2 == 0 else xb_d
            with (
                tc.tile_pool(name="h_sb", bufs=2) as hsb,
                tc.tile_pool(name="h_res", bufs=1) as hres,
                tc.tile_pool(name="h_ps", bufs=2, space="PSUM") as hps,
            ):
                u = hres.tile([128, 16 * Bc], F32)
                for k in range(16):
                    tok = 69 if k < 8 else 70
                    srcap = xfin[k % 8].rearrange("p (b t) -> p b t", t=T)[:, :, tok]
                    nc.sync.dma_start(u[:, k * Bc : (k + 1) * Bc], srcap)
                ub = hres.tile([128, 16 * Bc], BF16)
                for k in range(16):
                    ks = slice(k * Bc, (k + 1) * Bc)
                    nc.any.tensor_copy(ub[:, ks], u[:, ks])
                mean_ps = hps.tile([128, Bc], F32, tag="ln")
                for k in range(16):
                    nc.tensor.matmul(
                        mean_ps[:], ones128[:], ub[:, k * Bc : (k + 1) * Bc],
                        start=(k == 0), stop=(k == 15),
                    )
                sq_ps = hps.tile([128, Bc], F32, tag="ln")
                for k in range(16):
                    sqt = hsb.tile([128, Bc], BF16, tag="sq")
                    nc.scalar.activation(sqt[:], ub[:, k * Bc : (k + 1) * Bc], AF.Square)
                    nc.tensor.matmul(
                        sq_ps[:], ones128[:], sqt[:], start=(k == 0), stop=(k == 15)
                    )
                m1 = hsb.tile([128, Bc], F32, tag="m1")
                nc.vector.tensor_scalar_mul(m1[:], mean_ps[:], 1.0 / (2 * D))
                msq = hsb.tile([128, Bc], F32, tag="msq")
                nc.any.tensor_tensor(msq[:], m1[:], m1[:], ALU.mult)
                v = hsb.tile([128, Bc], F32, tag="v")
                nc.vector.scalar_tensor_tensor(
                    v[:], sq_ps[:], 1.0 / (2 * D), msq[:], ALU.mult, ALU.subtract
                )
                s = hsb.tile([128, Bc], F32, tag="s")
                nc.scalar.activation(s[:], v[:], AF.Sqrt, bias=epsT[:])
                r = hsb.tile([128, Bc], F32, tag="r")
                nc.vector.reciprocal(r[:], s[:])
                if flags["apply_ghead"]:
                    hg_sb = hres.tile([128, 16], F32)
                    nc.sync.dma_start(hg_sb[:], hg_d[:])
                    hbt_sb = hres.tile([128, 16], F32)
                    nc.sync.dma_start(hbt_sb[:], hbt_d[:])
                unb = hres.tile([128, 16 * Bc], BF16)
                for k in range(16):
                    ks = slice(k * Bc, (k + 1) * Bc)
                    xs = hsb.tile([128, Bc], F32, tag="xs")
                    nc.any.tensor_tensor(xs[:], u[:, ks], m1[:], ALU.subtract)
                    xn = hsb.tile([128, Bc], F32, tag="xn")
                    nc.any.tensor_tensor(xn[:], xs[:], r[:], ALU.mult)
                    if flags["apply_ghead"]:
                        nc.vector.tensor_scalar(
                            xn[:], xn[:], hg_sb[:, k : k + 1], hbt_sb[:, k : k + 1],
                            ALU.mult, ALU.add,
                        )
                    nc.any.tensor_copy(unb[:, ks], xn[:])
                # W1 + leaky
                w1_tiles = []
                for k in range(16):
                    wt = hres.tile([128, 2 * D], BF16, tag=f"w1_{k}")
                    nc.sync.dma_start(wt[:], w1t_d[k])
                    w1_tiles.append(wt)
                if flags["use_b1"]:
                    hb1_sb = hres.tile([128, 16], F32)
                    nc.sync.dma_start(hb1_sb[:], hb1_d[:])
                    hb1s_sb = hres.tile([128, 16], F32)
                    nc.sync.dma_start(hb1s_sb[:], hb1s_d[:])
                h1 = hres.tile([128, 16 * Bc], BF16)
                for m in range(16):
                    f_ps = hps.tile([128, Bc], F32, tag="f")
                    for k in range(16):
                        nc.tensor.matmul(
                            f_ps[:], w1_tiles[k][:, m * 128 : (m + 1) * 128],
                            unb[:, k * Bc : (k + 1) * Bc],
                            start=(k == 0), stop=(k == 15),
                        )
                    t1 = hsb.tile([128, Bc], F32, tag="t1")
                    ms = slice(m * Bc, (m + 1) * Bc)
                    if flags["use_b1"]:
                        nc.scalar.activation(
                            t1[:], f_ps[:], AF.Copy, scale=0.2, bias=hb1s_sb[:, m : m + 1]
                        )
                        s1 = hsb.tile([128, Bc], F32, tag="s1")
                        nc.vector.tensor_scalar_add(s1[:], f_ps[:], hb1_sb[:, m : m + 1])
                        nc.any.tensor_tensor(h1[:, ms], t1[:], s1[:], ALU.max)
                    else:
                        nc.scalar.activation(t1[:], f_ps[:], AF.Copy, scale=0.2)
                        nc.any.tensor_tensor(h1[:, ms], t1[:], f_ps[:], ALU.max)
                w2_sb = hres.tile([128, 16], BF16)
                nc.sync.dma_start(w2_sb[:], w2s_d[:])
                o_ps = hps.tile([1, Bc], F32, tag="o")
                for k in range(16):
                    nc.tensor.matmul(
                        o_ps[:], w2_sb[:, k : k + 1], h1[:, k * Bc : (k + 1) * Bc],
                        start=(k == 0), stop=(k == 15),
                    )
                o_sb = hsb.tile([1, Bc], F32, tag="o")
                if flags["use_b2"]:
                    b2_sb = hres.tile([1, 1], F32)
                    nc.sync.dma_start(b2_sb[:], b2_d[:])
                    nc.scalar.activation(o_sb[:], o_ps[:], AF.Sigmoid, bias=b2_sb[:])
                else:
                    nc.scalar.activation(o_sb[:], o_ps[:], AF.Sigmoid)
                nc.sync.dma_start(out_d[:], o_sb[:])

    return nc


TRACE = False
LAST_RESULT = None


def kernel(**inputs):
    global LAST_RESULT
    in_maps, flags = host_prep(inputs, N_CORES)
    nc = build_program(flags)
    nc.compile()
    res = run_bass_kernel_spmd(
        nc, in_maps, core_ids=list(range(N_CORES)), trace=TRACE
    )
    LAST_RESULT = res
    Bc = flags["Bc"]
    out = np.concatenate([res.results[c]["out"].reshape(Bc, 1) for c in range(N_CORES)])
    return out.astype(np.float32)

